# revision 3
# baseline (speedup 1.0000x reference)
"""OFT block-diagonal rotation forward (nn_Linear_12635793785535).

y = x @ blockdiag(rot_0..rot_63), rot_r = I + 2Q_r + 2Q_r^2 + 2Q_r^3 + 2Q_r^4
with Q_r the skew-symmetric matrix built from weight[r] (computed on host).

Sharding: data-parallel over tokens across 8 NeuronCores; the small derived
rotation pair-tiles are replicated (per the problem's sharding hint).

This problem is pure streaming (every x element read once, every y element
written once), so HW time == HBM traffic / bandwidth. Two levers vs the f32
row-major baseline (~33.6 MB/core, ~101 us):

1. fp16 I/O. The 2e-2 error budget is ~23x above the fp16 datapath's
   measured 8.6e-4, so x is staged to DRAM as fp16 and y returned as fp16
   (host up/down-converts). Traffic: 8 + 8 + 1 = 17 MB/core.
2. Host-side transpose. The PE contracts over features, which needs x with
   features on partitions. Instead of PE-transposing on device (which
   doubled PE work and burned PSUM/DVE), the host pre-lays x out as
   [128 part, 8 blk, 4 pair, 1024 tok] with part+pair = feature, so every
   DMA is 128 x 8KB fully-contiguous lines and the device does nothing but
   stationary-rot matmuls. y comes back in the same layout (out-channel on
   partitions) and the host inverts it.

Per core: 8 blocks x (1 MB x-in DMA, 8 matmuls n=512, 4 psum->sbuf fp16
copies, 1 MB y-out DMA). Queues: SP = x in (+ tail y halves), ACT = rot +
y out. Copies split 3:1 DVE:ACT. PE ~2.1 us/block and copies ~1.7 us/block
vs ~5.9 us/block of DMA: DMA is the only bottleneck, as it should be.
"""

import numpy as np

TOKENS = 8192
FEAT = 4096
R = 64
BLOCK = 64
NPAIR = 32  # pairs of 64-blocks -> 128-wide block-diagonal tiles
NUM_TERMS = 5
N_CORES = 8
TOK_SHARD = TOKENS // N_CORES  # 1024
BPAIR = 4  # pairs per DMA block
NBLK = NPAIR // BPAIR  # 8

_CACHE = {}

# test.py can flip these before calling kernel()
TRACE = False
LAST_RESULTS = None


def _build_bass():
    from contextlib import ExitStack

    import concourse.tile as tile
    from concourse import bacc, mybir

    nc = bacc.Bacc(
        "TRN2",
        target_bir_lowering=False,
        debug=False,
        enable_asserts=False,
        num_devices=N_CORES,
    )
    # x laid out on host as [part i, blk b, pair q, tok t] = xT[512b+128q+i, t]
    x_d = nc.dram_tensor(
        "x", [128, NBLK, BPAIR, TOK_SHARD], mybir.dt.float16, kind="ExternalInput"
    ).ap()
    # dense fp16 pair-tiles [k=128, pair, c=128]
    rot_d = nc.dram_tensor(
        "rot", [128, NPAIR, 128], mybir.dt.float16, kind="ExternalInput"
    ).ap()
    # y in the same [part, blk, pair, tok] layout (part = out-channel in pair)
    y_d = nc.dram_tensor(
        "y", [128, NBLK, BPAIR, TOK_SHARD], mybir.dt.float16, kind="ExternalOutput"
    ).ap()

    with tile.TileContext(nc) as tc, ExitStack() as ctx:
        const_pool = ctx.enter_context(tc.tile_pool(name="const", bufs=1))
        xpool = ctx.enter_context(tc.tile_pool(name="xin", bufs=1))
        ypool = ctx.enter_context(tc.tile_pool(name="yout", bufs=1))
        ps_pool = ctx.enter_context(tc.tile_pool(name="ps", bufs=4, space="PSUM"))

        # dummy 1-elem ACT op: absorbs the 1.28us ACT_TABLE_LOAD into the
        # preamble instead of the first y copy on the critical path
        warm = const_pool.tile([1, 1], mybir.dt.float32)
        nc.gpsimd.memset(warm[:], 0.0)
        nc.scalar.copy(warm[:], warm[:])

        rot_sb = const_pool.tile([128, NPAIR, 128], mybir.dt.float16)
        # ACT queue: tiny first chunk so block 0's matmuls start asap
        nc.scalar.dma_start(rot_sb[:, 0:BPAIR, :], rot_d[:, 0:BPAIR, :])
        nc.scalar.dma_start(rot_sb[:, BPAIR:NPAIR, :], rot_d[:, BPAIR:NPAIR, :])

        # Queue plan (each HWDGE queue sustains ~230 GB/s, ~440 GB/s agg;
        # balance is everything): SP = x even + y even + y7-half (8.5 MB),
        # ACT = rot + x odd + y odd + y7-half (8.5 MB). All x DMAs are
        # issued upfront (bufs=NBLK -> no write-after-read waits ever), so
        # both queues stream back-to-back from the preamble barrier on.
        def issue_x(b, fine=False):
            eng = nc.sync if b % 2 == 0 else nc.scalar
            xt = xpool.tile(
                [128, BPAIR, TOK_SHARD], mybir.dt.float16, name="xb", tag="xb",
                bufs=NBLK,
            )
            if fine:
                # per-pair chunks so the first matmul starts ~0.8us in
                for q in range(BPAIR):
                    eng.dma_start(xt[:, q, :], x_d[:, b, q, :])
            else:
                eng.dma_start(xt[:], x_d[:, b, :, :])
            return xt

        xb_tiles = [issue_x(0, fine=True)] + [issue_x(b) for b in range(1, NBLK)]

        for b in range(NBLK):
            xb = xb_tiles[b]
            yb = ypool.tile(
                [128, BPAIR, TOK_SHARD], mybir.dt.float16, name="yb", tag="yb",
                bufs=5,
            )
            for q in range(BPAIR):
                p = b * BPAIR + q
                ps = ps_pool.tile([128, TOK_SHARD], mybir.dt.float32, tag="ps", bufs=4)
                for h in range(2):
                    nc.tensor.matmul(
                        ps[:, h * 512 : (h + 1) * 512],
                        rot_sb[:, p, :],
                        xb[:, q, h * 512 : (h + 1) * 512],
                        start=True,
                        stop=True,
                    )
                # fp32 PSUM -> fp16 SBUF copies, 2:2 DVE:ACT (~1.2us DVE,
                # ~0.8us ACT per copy); ACT's y issue follows its own copies
                if q < 2:
                    nc.vector.tensor_copy(yb[:, q, :], ps[:])
                else:
                    nc.scalar.copy(yb[:, q, :], ps[:])
            if b == NBLK - 1:
                # tail: split the last block across both queues
                half = BPAIR // 2
                nc.sync.dma_start(y_d[:, b, 0:half, :], yb[:, 0:half, :])
                nc.scalar.dma_start(y_d[:, b, half:BPAIR, :], yb[:, half:BPAIR, :])
            else:
                eng = nc.sync if b % 2 == 0 else nc.scalar
                eng.dma_start(y_d[:, b, :, :], yb[:])

    nc.compile()
    return nc


def _host_rot_packed(weight):
    """Cayley-Neumann series on host (f32), laid out as dense fp16
    block-diagonal pair-tiles [k=128, pair, c=128] (replicated per core)."""
    w = np.asarray(weight, dtype=np.float32)
    rows, cols = np.triu_indices(BLOCK, k=1)
    Q = np.zeros((R, BLOCK, BLOCK), dtype=np.float32)
    Q[:, rows, cols] = w
    Q = Q - np.swapaxes(Q, 1, 2)
    eye = np.eye(BLOCK, dtype=np.float32)
    rot = eye[None, :, :] + 2.0 * Q
    Qp = Q
    for _ in range(2, NUM_TERMS):
        Qp = np.einsum("rij,rjk->rik", Qp, Q).astype(np.float32)
        rot = rot + 2.0 * Qp
    layout = np.zeros((128, NPAIR, 128), dtype=np.float32)
    for pair in range(NPAIR):
        layout[0:64, pair, 0:64] = rot[2 * pair]
        layout[64:128, pair, 64:128] = rot[2 * pair + 1]
    return layout.astype(np.float16)


def kernel(x, weight):
    global LAST_RESULTS
    if "nc" not in _CACHE:
        _CACHE["nc"] = _build_bass()
    nc = _CACHE["nc"]

    from concourse.bass_utils import run_bass_kernel_spmd

    xf16 = np.asarray(x, dtype=np.float16)
    rot = _host_rot_packed(weight)
    in_maps = []
    for i in range(N_CORES):
        sh = xf16[i * TOK_SHARD : (i + 1) * TOK_SHARD]  # [1024, 4096]
        lay = np.ascontiguousarray(
            sh.T.reshape(NBLK, BPAIR, 128, TOK_SHARD).transpose(2, 0, 1, 3)
        )
        in_maps.append({"x": lay, "rot": rot})
    res = run_bass_kernel_spmd(
        nc, in_maps, core_ids=list(range(N_CORES)), trace=TRACE
    )
    LAST_RESULTS = res
    outs = []
    for r in res.results:
        yT = np.asarray(r["y"]).transpose(1, 2, 0, 3).reshape(FEAT, TOK_SHARD)
        outs.append(np.ascontiguousarray(yT.T).astype(np.float32))
    return np.concatenate(outs, axis=0)


# revision 6
# speedup vs baseline: 1.0062x; 1.0062x over previous
"""OFT block-diagonal rotation forward (nn_Linear_12635793785535).

y = x @ blockdiag(rot_0..rot_63), rot_r = I + 2Q_r + 2Q_r^2 + 2Q_r^3 + 2Q_r^4
with Q_r the skew-symmetric matrix built from weight[r] (computed on host).

Sharding: data-parallel over tokens across 8 NeuronCores; the small derived
rotation pair-tiles are replicated (per the problem's sharding hint).

This problem is pure streaming (every x element read once, every y element
written once), so HW time == HBM traffic / bandwidth. Two levers vs the f32
row-major baseline (~33.6 MB/core, ~101 us):

1. fp16 I/O. The 2e-2 error budget is ~23x above the fp16 datapath's
   measured 8.6e-4, so x is staged to DRAM as fp16 and y returned as fp16
   (host up/down-converts). Traffic: 8 + 8 + 1 = 17 MB/core.
2. Host-side transpose. The PE contracts over features, which needs x with
   features on partitions. Instead of PE-transposing on device (which
   doubled PE work and burned PSUM/DVE), the host pre-lays x out as
   [128 part, 8 blk, 4 pair, 1024 tok] with part+pair = feature, so every
   DMA is 128 x 8KB fully-contiguous lines and the device does nothing but
   stationary-rot matmuls. y comes back in the same layout (out-channel on
   partitions) and the host inverts it.

Per core: 8 blocks x (1 MB x-in DMA, 8 matmuls n=512, 4 psum->sbuf fp16
copies, 1 MB y-out DMA). Queues: SP = x in (+ tail y halves), ACT = rot +
y out. Copies split 3:1 DVE:ACT. PE ~2.1 us/block and copies ~1.7 us/block
vs ~5.9 us/block of DMA: DMA is the only bottleneck, as it should be.
"""

import numpy as np

TOKENS = 8192
FEAT = 4096
R = 64
BLOCK = 64
NPAIR = 32  # pairs of 64-blocks -> 128-wide block-diagonal tiles
NUM_TERMS = 5
N_CORES = 8
TOK_SHARD = TOKENS // N_CORES  # 1024
BPAIR = 4  # pairs per DMA block
NBLK = NPAIR // BPAIR  # 8

_CACHE = {}

# test.py can flip these before calling kernel()
TRACE = False
LAST_RESULTS = None


def _build_bass():
    from contextlib import ExitStack

    import concourse.tile as tile
    from concourse import bacc, mybir

    nc = bacc.Bacc(
        "TRN2",
        target_bir_lowering=False,
        debug=False,
        enable_asserts=False,
        num_devices=N_CORES,
    )
    # x laid out on host as [part i, blk b, pair q, tok t] = xT[512b+128q+i, t]
    x_d = nc.dram_tensor(
        "x", [128, NBLK, BPAIR, TOK_SHARD], mybir.dt.float16, kind="ExternalInput"
    ).ap()
    # dense fp16 pair-tiles [k=128, pair, c=128]
    rot_d = nc.dram_tensor(
        "rot", [128, NPAIR, 128], mybir.dt.float16, kind="ExternalInput"
    ).ap()
    # y in the same [part, blk, pair, tok] layout (part = out-channel in pair)
    y_d = nc.dram_tensor(
        "y", [128, NBLK, BPAIR, TOK_SHARD], mybir.dt.float16, kind="ExternalOutput"
    ).ap()

    with tile.TileContext(nc) as tc, ExitStack() as ctx:
        const_pool = ctx.enter_context(tc.tile_pool(name="const", bufs=1))
        xpool = ctx.enter_context(tc.tile_pool(name="xin", bufs=1))
        ypool = ctx.enter_context(tc.tile_pool(name="yout", bufs=1))
        ps_pool = ctx.enter_context(tc.tile_pool(name="ps", bufs=4, space="PSUM"))

        # dummy 1-elem ACT op: absorbs the 1.28us ACT_TABLE_LOAD into the
        # preamble instead of the first y copy on the critical path
        warm = const_pool.tile([1, 1], mybir.dt.float32)
        nc.gpsimd.memset(warm[:], 0.0)
        nc.scalar.copy(warm[:], warm[:])

        rot_sb = const_pool.tile([128, NPAIR, 128], mybir.dt.float16)
        # ACT queue: tiny first chunk so block 0's matmuls start asap
        nc.scalar.dma_start(rot_sb[:, 0:BPAIR, :], rot_d[:, 0:BPAIR, :])
        nc.scalar.dma_start(rot_sb[:, BPAIR:NPAIR, :], rot_d[:, BPAIR:NPAIR, :])

        # Queue plan (each HWDGE queue sustains ~230 GB/s, ~440 GB/s agg).
        # Front-load ALL x reads on both queues so the last x block lands
        # ~31us in (the tail is gated by last-x + compute + last-y drain),
        # then drain the y backlog from deep SBUF rings:
        #   SP : x0(fine) x2 x4 x6 | y3 y4 y5 y6 y7-half   (8.5 MB)
        #   ACT: rot x1 x3 x5 x7   | y0 y1 y2    y7-half   (8.5 MB)
        # Two hard runtime constraints shape this: (1) only 8 DMA
        # completion-sem lanes exist, recycled round-robin, so a 9th
        # in-flight DMA's *issue op* stalls its engine until the 8-back DMA
        # drained -- the order above keeps every recycle wait pre-satisfied;
        # (2) y tile-ring reuse makes block b's copies wait on block
        # b-bufs's y DMA drain, so bufs=7 keeps that wait ahead of queue
        # demand. All x DMAs are issued upfront (bufs=NBLK, no WAR waits).
        def issue_x(b, fine=False):
            eng = nc.sync if b % 2 == 0 else nc.scalar
            xt = xpool.tile(
                [128, BPAIR, TOK_SHARD], mybir.dt.float16, name="xb", tag="xb",
                bufs=NBLK,
            )
            if fine:
                # per-pair chunks so the first matmul starts ~0.8us in
                for q in range(BPAIR):
                    eng.dma_start(xt[:, q, :], x_d[:, b, q, :])
            else:
                eng.dma_start(xt[:], x_d[:, b, :, :])
            return xt

        xb_tiles = [issue_x(0, fine=True)] + [issue_x(b) for b in range(1, NBLK)]

        for b in range(NBLK):
            xb = xb_tiles[b]
            yb = ypool.tile(
                [128, BPAIR, TOK_SHARD], mybir.dt.float16, name="yb", tag="yb",
                bufs=7,
            )
            for q in range(BPAIR):
                p = b * BPAIR + q
                ps = ps_pool.tile([128, TOK_SHARD], mybir.dt.float32, tag="ps", bufs=4)
                for h in range(2):
                    nc.tensor.matmul(
                        ps[:, h * 512 : (h + 1) * 512],
                        rot_sb[:, p, :],
                        xb[:, q, h * 512 : (h + 1) * 512],
                        start=True,
                        stop=True,
                    )
                # fp32 PSUM -> fp16 SBUF copies, 2:2 DVE:ACT (~1.2us DVE,
                # ~0.8us ACT per copy); ACT's y issue follows its own copies
                if q < 2:
                    nc.vector.tensor_copy(yb[:, q, :], ps[:])
                else:
                    nc.scalar.copy(yb[:, q, :], ps[:])
            if b == NBLK - 1:
                # tail: split the last block across both queues
                half = BPAIR // 2
                nc.sync.dma_start(y_d[:, b, 0:half, :], yb[:, 0:half, :])
                nc.scalar.dma_start(y_d[:, b, half:BPAIR, :], yb[:, half:BPAIR, :])
            else:
                eng = nc.scalar if b < 3 else nc.sync
                eng.dma_start(y_d[:, b, :, :], yb[:])

    nc.compile()
    return nc


def _host_rot_packed(weight):
    """Cayley-Neumann series on host (f32), laid out as dense fp16
    block-diagonal pair-tiles [k=128, pair, c=128] (replicated per core)."""
    w = np.asarray(weight, dtype=np.float32)
    rows, cols = np.triu_indices(BLOCK, k=1)
    Q = np.zeros((R, BLOCK, BLOCK), dtype=np.float32)
    Q[:, rows, cols] = w
    Q = Q - np.swapaxes(Q, 1, 2)
    eye = np.eye(BLOCK, dtype=np.float32)
    rot = eye[None, :, :] + 2.0 * Q
    Qp = Q
    for _ in range(2, NUM_TERMS):
        Qp = np.einsum("rij,rjk->rik", Qp, Q).astype(np.float32)
        rot = rot + 2.0 * Qp
    layout = np.zeros((128, NPAIR, 128), dtype=np.float32)
    for pair in range(NPAIR):
        layout[0:64, pair, 0:64] = rot[2 * pair]
        layout[64:128, pair, 64:128] = rot[2 * pair + 1]
    return layout.astype(np.float16)


def kernel(x, weight):
    global LAST_RESULTS
    if "nc" not in _CACHE:
        _CACHE["nc"] = _build_bass()
    nc = _CACHE["nc"]

    from concourse.bass_utils import run_bass_kernel_spmd

    xf16 = np.asarray(x, dtype=np.float16)
    rot = _host_rot_packed(weight)
    in_maps = []
    for i in range(N_CORES):
        sh = xf16[i * TOK_SHARD : (i + 1) * TOK_SHARD]  # [1024, 4096]
        lay = np.ascontiguousarray(
            sh.T.reshape(NBLK, BPAIR, 128, TOK_SHARD).transpose(2, 0, 1, 3)
        )
        in_maps.append({"x": lay, "rot": rot})
    res = run_bass_kernel_spmd(
        nc, in_maps, core_ids=list(range(N_CORES)), trace=TRACE
    )
    LAST_RESULTS = res
    outs = []
    for r in res.results:
        yT = np.asarray(r["y"]).transpose(1, 2, 0, 3).reshape(FEAT, TOK_SHARD)
        outs.append(np.ascontiguousarray(yT.T).astype(np.float32))
    return np.concatenate(outs, axis=0)


# revision 7
# speedup vs baseline: 1.1045x; 1.0976x over previous
"""OFT block-diagonal rotation forward (nn_Linear_12635793785535).

y = x @ blockdiag(rot_0..rot_63), rot_r = I + 2Q_r + 2Q_r^2 + 2Q_r^3 + 2Q_r^4
with Q_r the skew-symmetric matrix built from weight[r] (computed on host).

Sharding: data-parallel over tokens across 8 NeuronCores; the small derived
rotation pair-tiles are replicated (per the problem's sharding hint).

Pure streaming problem: HW time == HBM traffic / bandwidth. Levers vs the
f32 row-major baseline (33.6 MB/core, ~101 us):

1. fp16 I/O (error budget 2e-2 vs measured 8.6e-4): x staged to DRAM as
   fp16, y returned as fp16; host up/down-converts. 17 MB/core total.
2. Host-side transpose: x is pre-laid-out as [128 part, blk, pair, tok]
   with part+pair = feature, so every DMA is long contiguous lines and the
   PE does nothing but stationary-rot matmuls (no on-device transposes,
   no dtype converts). y comes back in the same layout and is inverted on
   the host.

Scheduling facts this kernel is built around (measured via ntff traces):
- The two HWDGE queues (SP, ACT) share ~440 GB/s aggregate; each runs
  ~220 GB/s when co-busy. Keeping BOTH continuously fed is everything.
- Every dma_start burns one of 8 completion-semaphore lanes, recycled
  round-robin globally. Lane reuse makes the *issue op* wait for the
  8-back DMA; a third use makes the Tile scheduler emit a reset-rendezvous
  op (waits on consumer progress counters) that can block its engine for
  ~10 us. So: FEW, BIG DMAs (14 total), and nothing data-dependent queued
  behind a potential reset on the same engine.
- A dma_start costs 0.6-1 us of sequencer issue time -> also favors few.
- Everything is fully SBUF-resident (x 64K + y 64K + rot 8K per
  partition), so no tile-ring write-after-read waits exist at all.

Layout: SP queue = all of x (x0 as two 2-pair chunks so matmuls start
~11 us in, then three 2-block DMAs) + y6 + y7-half at the tail. ACT queue
= rot (tiny head chunk first) + x7 early (fills the gap while y0 isn't
ready yet) + y as three 2-block DMAs + y7-half. fp32 PSUM -> fp16 SBUF
copies split 2:2 between DVE (~1.2 us) and ACT (~1.1 us) per block.
"""

import numpy as np

TOKENS = 8192
FEAT = 4096
R = 64
BLOCK = 64
NPAIR = 32  # pairs of 64-blocks -> 128-wide block-diagonal tiles
NUM_TERMS = 5
N_CORES = 8
TOK_SHARD = TOKENS // N_CORES  # 1024
BPAIR = 4  # pairs per block (1 MB)
NBLK = NPAIR // BPAIR  # 8

_CACHE = {}

# test.py can flip these before calling kernel()
TRACE = False
LAST_RESULTS = None


def _build_bass():
    from contextlib import ExitStack

    import concourse.tile as tile
    from concourse import bacc, mybir

    nc = bacc.Bacc(
        "TRN2",
        target_bir_lowering=False,
        debug=False,
        enable_asserts=False,
        num_devices=N_CORES,
    )
    # x laid out on host as [part i, blk b, pair q, tok t] = xT[512b+128q+i, t]
    x_d = nc.dram_tensor(
        "x", [128, NBLK, BPAIR, TOK_SHARD], mybir.dt.float16, kind="ExternalInput"
    ).ap()
    # dense fp16 pair-tiles [k=128, pair, c=128]
    rot_d = nc.dram_tensor(
        "rot", [128, NPAIR, 128], mybir.dt.float16, kind="ExternalInput"
    ).ap()
    # y in the same [part, blk, pair, tok] layout (part = out-channel in pair)
    y_d = nc.dram_tensor(
        "y", [128, NBLK, BPAIR, TOK_SHARD], mybir.dt.float16, kind="ExternalOutput"
    ).ap()

    f16 = mybir.dt.float16

    with tile.TileContext(nc) as tc, ExitStack() as ctx:
        const_pool = ctx.enter_context(tc.tile_pool(name="const", bufs=1))
        xpool = ctx.enter_context(tc.tile_pool(name="xin", bufs=1))
        ypool = ctx.enter_context(tc.tile_pool(name="yout", bufs=1))
        ps_pool = ctx.enter_context(tc.tile_pool(name="ps", bufs=4, space="PSUM"))

        # dummy 1-elem ACT op: absorbs the 1.28us ACT_TABLE_LOAD into the
        # preamble instead of the first y copy on the critical path
        warm = const_pool.tile([1, 1], mybir.dt.float32)
        nc.gpsimd.memset(warm[:], 0.0)
        nc.scalar.copy(warm[:], warm[:])

        rot_sb = const_pool.tile([128, NPAIR, 128], f16)
        nc.scalar.dma_start(rot_sb[:, 0:BPAIR, :], rot_d[:, 0:BPAIR, :])
        nc.scalar.dma_start(rot_sb[:, BPAIR:NPAIR, :], rot_d[:, BPAIR:NPAIR, :])

        # ---- x in: SP = x0 (two 2-pair chunks) + x1-2, x3-4, x5-6;
        #      ACT = x7 (early, keeps ACT busy while no y is ready yet)
        x0 = xpool.tile([128, 1, BPAIR, TOK_SHARD], f16, name="x0")
        nc.sync.dma_start(x0[:, :, 0:2, :], x_d[:, 0:1, 0:2, :])
        nc.sync.dma_start(x0[:, :, 2:4, :], x_d[:, 0:1, 2:4, :])
        x12 = xpool.tile([128, 2, BPAIR, TOK_SHARD], f16, name="x12")
        nc.sync.dma_start(x12[:], x_d[:, 1:3, :, :])
        x34 = xpool.tile([128, 2, BPAIR, TOK_SHARD], f16, name="x34")
        nc.sync.dma_start(x34[:], x_d[:, 3:5, :, :])
        x56 = xpool.tile([128, 2, BPAIR, TOK_SHARD], f16, name="x56")
        nc.sync.dma_start(x56[:], x_d[:, 5:7, :, :])
        x7 = xpool.tile([128, 1, BPAIR, TOK_SHARD], f16, name="x7")
        nc.scalar.dma_start(x7[:], x_d[:, 7:8, :, :])

        def xblk(b):  # -> (tile, index within tile)
            if b == 0:
                return x0, 0
            if b == 7:
                return x7, 0
            return (x12, x34, x56)[(b - 1) // 2], (b - 1) % 2

        # ---- y out: double-block tiles, ACT = y01, y23, y45; tail y6 on
        #      SP, y7 split across both queues
        ybig = [
            ypool.tile([128, 2, BPAIR, TOK_SHARD], f16, name=f"y{2*k}{2*k+1}")
            for k in range(4)
        ]

        for b in range(NBLK):
            xt, xi = xblk(b)
            yb = ybig[b // 2]
            yi = b % 2
            for q in range(BPAIR):
                p = b * BPAIR + q
                ps = ps_pool.tile([128, TOK_SHARD], mybir.dt.float32, tag="ps", bufs=4)
                for h in range(2):
                    nc.tensor.matmul(
                        ps[:, h * 512 : (h + 1) * 512],
                        rot_sb[:, p, :],
                        xt[:, xi, q, h * 512 : (h + 1) * 512],
                        start=True,
                        stop=True,
                    )
                if q < 2:
                    nc.vector.tensor_copy(yb[:, yi, q, :], ps[:])
                else:
                    nc.scalar.copy(yb[:, yi, q, :], ps[:])
            if b % 2 == 1 and b < 6:
                nc.scalar.dma_start(y_d[:, b - 1 : b + 1, :, :], ybig[b // 2][:])
            elif b == 6:
                nc.sync.dma_start(y_d[:, 6:7, :, :], ybig[3][:, 0:1, :, :])
            elif b == 7:
                nc.sync.dma_start(y_d[:, 7:8, 0:2, :], ybig[3][:, 1:2, 0:2, :])
                nc.scalar.dma_start(y_d[:, 7:8, 2:4, :], ybig[3][:, 1:2, 2:4, :])

    nc.compile()
    return nc


def _host_rot_packed(weight):
    """Cayley-Neumann series on host (f32), laid out as dense fp16
    block-diagonal pair-tiles [k=128, pair, c=128] (replicated per core)."""
    w = np.asarray(weight, dtype=np.float32)
    rows, cols = np.triu_indices(BLOCK, k=1)
    Q = np.zeros((R, BLOCK, BLOCK), dtype=np.float32)
    Q[:, rows, cols] = w
    Q = Q - np.swapaxes(Q, 1, 2)
    eye = np.eye(BLOCK, dtype=np.float32)
    rot = eye[None, :, :] + 2.0 * Q
    Qp = Q
    for _ in range(2, NUM_TERMS):
        Qp = np.einsum("rij,rjk->rik", Qp, Q).astype(np.float32)
        rot = rot + 2.0 * Qp
    layout = np.zeros((128, NPAIR, 128), dtype=np.float32)
    for pair in range(NPAIR):
        layout[0:64, pair, 0:64] = rot[2 * pair]
        layout[64:128, pair, 64:128] = rot[2 * pair + 1]
    return layout.astype(np.float16)


def kernel(x, weight):
    global LAST_RESULTS
    if "nc" not in _CACHE:
        _CACHE["nc"] = _build_bass()
    nc = _CACHE["nc"]

    from concourse.bass_utils import run_bass_kernel_spmd

    xf16 = np.asarray(x, dtype=np.float16)
    rot = _host_rot_packed(weight)
    in_maps = []
    for i in range(N_CORES):
        sh = xf16[i * TOK_SHARD : (i + 1) * TOK_SHARD]  # [1024, 4096]
        lay = np.ascontiguousarray(
            sh.T.reshape(NBLK, BPAIR, 128, TOK_SHARD).transpose(2, 0, 1, 3)
        )
        in_maps.append({"x": lay, "rot": rot})
    res = run_bass_kernel_spmd(
        nc, in_maps, core_ids=list(range(N_CORES)), trace=TRACE
    )
    LAST_RESULTS = res
    outs = []
    for r in res.results:
        yT = np.asarray(r["y"]).transpose(1, 2, 0, 3).reshape(FEAT, TOK_SHARD)
        outs.append(np.ascontiguousarray(yT.T).astype(np.float32))
    return np.concatenate(outs, axis=0)
